# revision 4
# baseline (speedup 1.0000x reference)
"""Bass/Trainium2 kernel for nn_AdaptiveSparseReservoir (self-contained).

out[b, c] = relu(sum_k x[b, rows[k]] * values[k] for cols[k]==c  + bias[c])
  x [1024, 4096] f32; values [262144] f32; rows/cols [262144] i32;
  bias [4096] f32  ->  out [1024, 4096] f32

Strategy
--------
Densify the sparse COO kernel on the host into W [4096, 4096] (1.6%
density with unstructured support is far too dense for gather/scatter on
TRN2 — a dense bf16 TensorEngine matmul moves ~16x fewer bytes), then run
the dense matmul column-sharded across the 8 NeuronCores with NO
collectives: core i computes outT_i = relu(W[:, 512i:512(i+1)].T @ x.T + b_i).

Measured-on-silicon design points:
- PSUM-accumulating bf16 matmuls (K=128, N=512) retire at ~213-226 ns —
  one rhs column per 2.4 GHz cycle is the architectural floor (~55 us for
  the 256 matmuls/core; fp8 DoubleRow would halve it but its ~5% quant
  error fails the 2e-2 gate; no int8 matmul path exists on TRN2 bass).
  DMA (12 MB/core) streams at ~310 GB/s split across BOTH HWDGE rings
  (sync + scalar) and hides under the matmul stream: "ridge" regime.
- The output is computed TRANSPOSED so the per-column bias lands on the
  PSUM partition axis: bias+relu is then a single fused op per PSUM bank,
  alternating ScalarE `activation` / VectorE `tensor_scalar`.
- STAGGERED DRAIN (-3 us/body vs the clustered-epilogue baseline, median
  of seven interleaved A/B sessions): the last 4 k-tiles run
  bank-pair-major, so the 8 bank completions spread over the final
  ~6.8 us of the matmul stream and every bias+relu epilogue + out-DMA
  pipelines UNDER the stream instead of serializing after it. The final
  pair's epilogues are halved across both engines and the out-DMAs
  balanced across both rings, leaving one 256-col epilogue + one 64 KB
  DMA exposed. PSUM collisions are per-BANK (an epilogue read blocks
  further matmuls to that bank), so bank granularity is the floor here.
- EARLY START: the DMA head is split so the first matmul — an N=256 half
  of bank (0,0) — waits only on a 64 KB x chunk + 32 KB w chunk on
  parallel rings (~0.5 us). start=True clears has_written for the WHOLE
  bank, so only the first half sets it; the second half runs start=False
  and overwrites its cleared region (verified on silicon). The chunk head
  is k-granular (no PE stall in an arrival-vs-consumption model even at
  120 GB/s/ring); the bias load rides behind the stream. Short N=128
  warm-up matmuls cover the HAM cold-clock window until data lands.
- TileContext's exit barrier is replaced by a drain-only tail: the Bass
  preamble sem_clears at the start of every execution, so the butterfly
  barrier + semaphore clears (~4 us) are dead weight.
- Ruled out on silicon: coarse DMA chunking with greedy ring balancing
  (+712 ns, descriptor count immaterial) and 4-k-contiguous bank-major
  matmul order (+1020 ns, psum bank round-robin costs nothing here).
"""

import os
import types

import numpy as np
import ml_dtypes

D_IN = 4096
UNITS = 4096
NNZ = 262144
BATCH = 1024
N_CORES = 8
N_SHARD = UNITS // N_CORES  # 512 output columns per core
K_TILES = D_IN // 128  # 32
N_TILES = N_SHARD // 128  # 4
M_HALVES = BATCH // 512  # 2
N_WARMUP = 3

_CACHE = {}


def _drain_only(self, tick_clock, wait_clock):
    """Tail = DMA/compute drain only; skip the butterfly barrier + sem
    clears (the Bass preamble sem_clears at the start of each execution,
    and NEFF completion already requires every engine queue to finish)."""
    from concourse.tile import ScopedClock

    drain_inst = self.nc.sync.drain()
    wait_clock.add_sem_waits(
        drain_inst.ins, ScopedClock({None: tick_clock.global_clock})
    )
    popped = self.nc._tile_sem_poison_stack.pop()
    assert popped is self._sem_poison


def _build(reps=1):
    import concourse.mybir as mybir
    import concourse.tile as tile
    from concourse import bacc

    nc = bacc.Bacc("TRN2", target_bir_lowering=False, debug=False, num_devices=N_CORES)
    bf16 = mybir.dt.bfloat16
    f32 = mybir.dt.float32

    xT_ext = nc.declare_dram_parameter("xT", [128, K_TILES * 1024], bf16, isOutput=False)
    w_ext = nc.declare_dram_parameter("w", [128, K_TILES * 512], bf16, isOutput=False)
    b_ext = nc.declare_dram_parameter("bias", [128, N_TILES], f32, isOutput=False)
    out_ext = nc.declare_dram_parameter("out", [N_SHARD, BATCH], bf16, isOutput=True)

    tc_outer = tile.TileContext(nc)
    try:
        # verify the internals _drain_only touches exist in this concourse
        from concourse.tile import ScopedClock  # noqa: F401

        assert hasattr(tc_outer, "_drain_and_barrier")
        assert hasattr(nc, "_tile_sem_poison_stack")
        tc_outer._drain_and_barrier = types.MethodType(_drain_only, tc_outer)
    except Exception:
        pass  # stock barrier exit: ~4us slower, still correct
    with tc_outer as tc:
        with (
            tc.tile_pool(name="consts", bufs=1) as cpool,
            tc.tile_pool(name="xk", bufs=1) as xpool,
            tc.tile_pool(name="wk", bufs=1) as wpool,
            tc.tile_pool(name="osb", bufs=9) as opool,
            tc.tile_pool(name="psum", bufs=1, space="PSUM") as ppool,
        ):
            psum = [
                ppool.tile([128, 512], f32, tag=f"ps{i}", name=f"ps{i}")
                for i in range(N_TILES * M_HALVES)
            ]

            # PE warm-up against the HAM cold clock: short N=128 matmuls keep
            # the PE-busy window covered until the first data chunk lands,
            # with at most ~107 ns of overshoot past data arrival (an N=512
            # warm-up would block the queue 427 ns at the cold rate). k=0's
            # start=True clear discards the garbage.
            warm = cpool.tile([128, 128], bf16)
            nc.vector.memset(warm[:, :], 0)
            for _ in range(8):
                # lhsT and rhs may share the region: both are SBUF reads
                nc.tensor.matmul(
                    psum[0][:, :128], warm[:, 0:128], warm[:, 0:128],
                    start=True, stop=True,
                )

            tbl_warm = cpool.tile([128, 1], f32)
            bias_sb = cpool.tile([128, N_TILES], f32)

            xts = xpool.tile([128, K_TILES * 1024], bf16, name="xts")
            wts = wpool.tile([128, K_TILES * 512], bf16, name="wts")

            # interleave x/w chunks in k order, alternating HWDGE rings;
            # fine-grained first chunks (early PE start). The head is split
            # so the FIRST matmul — an N=256 half of bank (0,0) — waits only
            # on a 64 KB x transfer and a 32 KB w transfer on parallel
            # rings (~0.5 us), instead of two 128 KB ones.
            chunks = [
                ("x", 0, 256), ("w", 0, 128), ("x", 256, 512),
                ("w", 128, 512), ("x", 512, 1024), ("w", 512, 1024),
            ]
            xbounds = [1, 2, 3, 4, 5, 6] + list(range(8, K_TILES + 1, 2))
            wbounds = [2, 3, 4, 6, 8] + list(range(12, K_TILES + 1, 4))
            xi = wi = 0
            while xi < len(xbounds) - 1 or wi < len(wbounds) - 1:
                kx = xbounds[xi] if xi < len(xbounds) - 1 else K_TILES
                kw = wbounds[wi] if wi < len(wbounds) - 1 else K_TILES
                if kw < kx and wi < len(wbounds) - 1:
                    chunks.append(("w", wbounds[wi] * 512, wbounds[wi + 1] * 512))
                    wi += 1
                else:
                    chunks.append(("x", xbounds[xi] * 1024, xbounds[xi + 1] * 1024))
                    xi += 1

            def mm(k, nt, mh, stop=False):
                nc.tensor.matmul(
                    psum[nt * M_HALVES + mh][:, :],
                    wts[:, k * 512 + nt * 128 : k * 512 + (nt + 1) * 128],
                    xts[:, k * 1024 + mh * 512 : k * 1024 + (mh + 1) * 512],
                    start=(k == 0),
                    stop=stop,
                )

            def epilogue(r, i, nt, mh, lo, hi, on_scalar):
                ot = opool.tile([128, hi - lo], bf16, name=f"ot{r}_{i}_{lo}", tag="ot")
                if on_scalar:
                    nc.scalar.activation(
                        ot[:, :],
                        psum[nt * M_HALVES + mh][:, lo:hi],
                        mybir.ActivationFunctionType.Relu,
                        bias=bias_sb[:, nt : nt + 1],
                    )
                else:
                    nc.vector.tensor_scalar(
                        ot[:, :],
                        psum[nt * M_HALVES + mh][:, lo:hi],
                        bias_sb[:, nt : nt + 1],
                        0.0,
                        mybir.AluOpType.add,
                        mybir.AluOpType.max,
                    )
                return ot

            K_STAG = K_TILES - 4
            for r in range(reps):
                for i, (kind, clo, chi) in enumerate(chunks):
                    eng = nc.sync if i % 2 == 0 else nc.scalar
                    if kind == "x":
                        eng.dma_start(xts[:, clo:chi], xT_ext[:, clo:chi])
                    else:
                        eng.dma_start(wts[:, clo:chi], w_ext[:, clo:chi])

                if r == 0:
                    # bias (2 KB) rides behind the input stream — needed
                    # only by the first epilogue ~6 us before stream end
                    nc.sync.dma_start(bias_sb[:, :], b_ext[:, :])
                    # trigger the Relu act-table load now (ACT is idle during
                    # the stream); bacc hoists LoadActFuncSet before this
                    # instruction, keeping the ~1.3us load off the epilogue
                    # critical path
                    nc.scalar.activation(
                        tbl_warm[:, :], warm[:, 0:1],
                        mybir.ActivationFunctionType.Relu,
                    )

                # k=0, mh=0 first across all nt (only x cols 0:512 needed);
                # bank (0,0) starts as two N=256 halves gated on just the
                # first 64 KB x chunk. start=True clears has_written for the
                # WHOLE bank, so only the first half sets it; the second
                # half runs start=False and overwrites its cleared region.
                nc.tensor.matmul(
                    psum[0][:, 0:256], wts[:, 0:128], xts[:, 0:256],
                    start=True, stop=False,
                )
                nc.tensor.matmul(
                    psum[0][:, 256:512], wts[:, 0:128], xts[:, 256:512],
                    start=False, stop=False,
                )
                for nt in range(1, N_TILES):
                    mm(0, nt, 0)
                for nt in range(N_TILES):
                    mm(0, nt, 1)

                # phase 1: k-major over all banks — keeps the PE stream dense
                # while DMA feeds k-tiles. mh-inner so each LDWEIGHTS serves
                # two matmuls.
                for k in range(1, K_STAG):
                    for nt in range(N_TILES):
                        for mh in range(M_HALVES):
                            mm(k, nt, mh)

                # phase 2: bank-pair-major — each nt runs its last 4 k-tiles
                # (mh pairs share LDWEIGHTS) then both banks drain (fused
                # bias+relu, alternating ScalarE/VectorE; out-DMA alternating
                # rings). Pair completions are staggered 8 MMs (~1.7 us)
                # apart, so all 8 epilogues + out-DMAs pipeline UNDER the
                # remaining matmul stream instead of serializing after it.
                # The final pair's epilogues are split into halves across
                # both engines and both rings to shorten the exposed tail.
                for nt in range(N_TILES - 1):
                    for k in range(K_STAG, K_TILES):
                        mm(k, nt, 0, stop=(k == K_TILES - 1))
                        mm(k, nt, 1, stop=(k == K_TILES - 1))
                    for mh in range(M_HALVES):
                        i = nt * M_HALVES + mh
                        orow = out_ext[
                            nt * 128 : (nt + 1) * 128, mh * 512 : (mh + 1) * 512
                        ]
                        ot = epilogue(r, i, nt, mh, 0, 512, on_scalar=(i % 2 == 0))
                        eng = nc.sync if i % 2 == 0 else nc.scalar
                        eng.dma_start(orow, ot[:, :])

                # final pair (nt=3): each bank's epilogue is halved across
                # ACT (lo) and DVE (hi), and the four 64 KB out-DMAs are
                # balanced lo->sync / hi->scalar so both rings drain two each
                # in parallel. PSUM collisions are per-bank, so nothing can
                # start a bank's epilogue before its last matmul — this is
                # the minimal exposed tail at bank granularity.
                nt = N_TILES - 1
                for k in range(K_STAG, K_TILES):
                    mm(k, nt, 0, stop=(k == K_TILES - 1))
                    mm(k, nt, 1, stop=(k == K_TILES - 1))
                for mh in range(M_HALVES):
                    orow = out_ext[
                        nt * 128 : (nt + 1) * 128, mh * 512 : (mh + 1) * 512
                    ]
                    i = nt * M_HALVES + mh
                    ot0 = epilogue(r, i, nt, mh, 0, 256, on_scalar=True)
                    ot1 = epilogue(r, i, nt, mh, 256, 512, on_scalar=False)
                    nc.sync.dma_start(orow[:, 0:256], ot0[:, :])
                    nc.scalar.dma_start(orow[:, 256:512], ot1[:, :])

    nc.compile()
    return nc


def _get_nc():
    if "nc" not in _CACHE:
        _CACHE["nc"] = _build()
    return _CACHE["nc"]


def prep_in_maps(x, values, bias, rows, cols):
    x = np.asarray(x, np.float32)
    values = np.asarray(values, np.float32)
    bias = np.asarray(bias, np.float32)
    rows = np.asarray(rows)
    cols = np.asarray(cols)

    # densify via bincount (vectorized scatter-add; duplicates accumulate)
    flat = rows.astype(np.int64) * UNITS + cols.astype(np.int64)
    W = np.bincount(flat, weights=values.astype(np.float64), minlength=D_IN * UNITS)
    W = W.reshape(D_IN, UNITS).astype(np.float32)

    # partition-major xT: xT_pm[p, k*1024 + m] = x[m, k*128 + p]
    xT16 = np.ascontiguousarray(x.T).astype(ml_dtypes.bfloat16)  # [D_IN, BATCH]
    xT_pm = np.ascontiguousarray(
        xT16.reshape(K_TILES, 128, BATCH).transpose(1, 0, 2).reshape(128, K_TILES * BATCH)
    )
    W16 = W.astype(ml_dtypes.bfloat16)

    in_maps = []
    for i in range(N_CORES):
        w_shard = W16[:, i * N_SHARD : (i + 1) * N_SHARD]  # [D_IN, 512]
        # partition-major W: w_pm[p, k*512 + n] = W[k*128 + p, n0 + n]
        w_pm = np.ascontiguousarray(
            w_shard.reshape(K_TILES, 128, N_SHARD)
            .transpose(1, 0, 2)
            .reshape(128, K_TILES * N_SHARD)
        )
        b_shard = np.ascontiguousarray(
            bias[i * N_SHARD : (i + 1) * N_SHARD].reshape(N_TILES, 128).T
        )
        in_maps.append({"xT": xT_pm, "w": w_pm, "bias": b_shard})
    return in_maps


def kernel(x, values, bias, rows, cols):
    from concourse.bass_utils import run_bass_kernel_spmd

    in_maps = prep_in_maps(x, values, bias, rows, cols)
    nc = _get_nc()
    res = run_bass_kernel_spmd(nc, in_maps, list(range(N_CORES)))
    out = np.empty((BATCH, UNITS), np.float32)
    for i in range(N_CORES):
        out[:, i * N_SHARD : (i + 1) * N_SHARD] = (
            res.results[i]["out"].astype(np.float32).T
        )
    return out

